# revision 6
# baseline (speedup 1.0000x reference)
"""Trainium2 Bass kernel for batched multi-head graph attention (GAT).

Reference computation (per batch b, head h):
    h_prime = h[b] @ w[h]                      # [N, FOUT]
    t = tanh(h_prime)
    src = t @ a_src[h]; dst = t @ a_dst[h]     # [N]
    s[i,j] = leaky_relu_{0.2}(src[i] + dst[j])
    attn = softmax_j(where(adj[b]>0, s, -inf))
    out[b,h] = attn @ h_prime

Device algorithm (core c <-> batch b=c):
    exp(leaky_relu(s)) = max(e^s, e^{0.2 s}); with s = src_i + dst_j the
    unnormalized weight factors as
        W[j,i] = adjT[j,i] * e^{src_i} * max(u_i * vq_j, q_j)
    with u = e^{-0.8 src}, vq = e^{0.2 dst}, q = e^{dst}. The e^{src_i}
    factor is row-constant so it cancels in the softmax and is never
    computed. Per 128-row chunk of the score matrix only two DVE ops run:
        mxq = tensor_scalar(ub, *vq_j, *q_j, mult, max)   (4x rate, bf16)
        Z   = tensor_tensor(mxq, adjT, mult)              (2x rate, bf16)
    and the PE contracts outT[f,i] += hp1[j,f] * Z[j,i] where
    hp1 = [h_prime | 1]; the ones column accumulates the softmax
    denominator. Projections src/dst are computed on the PE from a
    head-pair-packed transposed h_prime (hpT = w^T h), so the vector
    engine does no projection work. The host divides rows 0..63 by row 64
    and transposes to [b, h, n, f].
"""

import numpy as np
import ml_dtypes

import concourse.mybir as mybir
import concourse.tile as tile
from concourse import bacc
from concourse.bass_utils import run_bass_kernel_spmd

BS, N, FIN, NH, FOUT = 8, 1024, 256, 8, 64
P = 128
NCH = N // P          # 8 chunks of the node axis
KC = FIN // P         # 2 chunks of the feature-in axis
NPAIR = NH // 2       # head pairs packed into 128 partitions
F32 = mybir.dt.float32
BF16 = mybir.dt.bfloat16
AX = mybir.AxisListType
ALU = mybir.AluOpType
ACTF = mybir.ActivationFunctionType
BF16NP = ml_dtypes.bfloat16


def emit(nc, tc, hT_d, w_d, aPC_d, adjT_d, out_d):
    with (
        tc.tile_pool(name="const", bufs=1) as cpool,
        tc.tile_pool(name="ub", bufs=3) as ubpool,
        tc.tile_pool(name="mx", bufs=6) as mxpool,
        tc.tile_pool(name="z", bufs=6) as zpool,
        tc.tile_pool(name="osb", bufs=2) as opool,
        tc.tile_pool(name="psa", bufs=2, space="PSUM") as pp_a,
        tc.tile_pool(name="psb", bufs=2, space="PSUM") as pp_b,
        tc.tile_pool(name="psout", bufs=4, space="PSUM") as pp_out,
    ):
        # ---- constant loads ----
        hT = cpool.tile([P, KC, N], BF16)
        wsb = cpool.tile([P, KC, NH * FOUT], BF16)
        aPC = cpool.tile([P, NPAIR, 4], BF16)
        adjT = cpool.tile([P, NCH, N], BF16)
        nc.sync.dma_start(wsb[:, 0, :], w_d[0])
        nc.sync.dma_start(wsb[:, 1, :], w_d[1])
        for kc in range(KC):
            for hf in range(2):
                nc.sync.dma_start(
                    hT[:, kc, hf * 512 : (hf + 1) * 512],
                    hT_d[kc, :, hf * 512 : (hf + 1) * 512],
                )
        for p in range(NPAIR):
            nc.sync.dma_start(aPC[:, p, :], aPC_d[p])
        for jc in range(NCH):
            nc.sync.dma_start(adjT[:, jc, :], adjT_d[jc])
        ones_row = cpool.tile([1, P], BF16)
        nc.vector.memset(ones_row, 1.0)

        # hp1[:, ic, h, 0:64] = h_prime, col 64 = ones (softmax denominator)
        hp1 = cpool.tile([P, NCH, NH, FOUT + 1], BF16)
        nc.vector.memset(hp1[:, :, :, FOUT : FOUT + 1], 1.0)
        tT = cpool.tile([P, NPAIR, N], BF16)
        u_sb = [cpool.tile([1, N], BF16, name=f"u{h}") for h in range(NH)]
        q_col = cpool.tile([P, NPAIR, NCH, 2], F32)
        vq_col = cpool.tile([P, NPAIR, NCH, 2], F32)

        # ---- phase A-T: hpT = w^T h (head pairs on partitions), tanh,
        #      then src (rows) and dst (cols) projections on the PE ----
        for p in range(NPAIR):
            for half in range(2):
                hpT = pp_b.tile([P, 512], F32, tag="b")
                for kc in range(KC):
                    nc.tensor.matmul(
                        hpT,
                        wsb[:, kc, 2 * p * FOUT : (2 * p + 2) * FOUT],
                        hT[:, kc, half * 512 : (half + 1) * 512],
                        start=(kc == 0),
                        stop=(kc == KC - 1),
                    )
                nc.scalar.activation(
                    tT[:, p, half * 512 : (half + 1) * 512], hpT, ACTF.Tanh
                )
            # src rows: one 1-col matmul per head so the PSUM read starts
            # at partition 0 (the verifier rejects offset partition reads)
            for half in range(2):
                for h2 in range(2):
                    srow = pp_a.tile([1, 512], F32, tag="a")
                    nc.tensor.matmul(
                        srow,
                        aPC[:, p, h2 : h2 + 1],
                        tT[:, p, half * 512 : (half + 1) * 512],
                        start=True,
                        stop=True,
                    )
                    nc.scalar.activation(
                        u_sb[2 * p + h2][0:1, half * 512 : (half + 1) * 512],
                        srow,
                        ACTF.Exp, scale=-0.8,
                    )
            # dst cols: [128, 2] per jc = tT_chunk^T @ aDstCols
            dcol = pp_a.tile([P, NCH * 2], F32, tag="a")
            for jc in range(NCH):
                nc.tensor.matmul(
                    dcol[:, 2 * jc : 2 * jc + 2],
                    tT[:, p, jc * P : (jc + 1) * P],
                    aPC[:, p, 2:4],
                    start=True,
                    stop=True,
                )
            nc.scalar.activation(
                q_col[:, p, :, :], dcol.rearrange("p (j c) -> p j c", c=2), ACTF.Exp
            )
            nc.scalar.activation(
                vq_col[:, p, :, :], dcol.rearrange("p (j c) -> p j c", c=2),
                ACTF.Exp, scale=0.2,
            )

        # ---- phase A: h_prime in [node, head*fout] layout for stationaries ----
        for ic in range(NCH):
            ps = pp_a.tile([P, NH * FOUT], F32, tag="a")
            for kc in range(KC):
                nc.tensor.matmul(
                    ps,
                    hT[:, kc, ic * P : (ic + 1) * P],
                    wsb[:, kc, :],
                    start=(kc == 0),
                    stop=(kc == KC - 1),
                )
            nc.scalar.activation(
                hp1[:, ic, :, 0:FOUT],
                ps.rearrange("p (h f) -> p h f", f=FOUT),
                ACTF.Copy,
            )

        # ---- phase C: masked weights + attention matmuls ----
        for h in range(NH):
            p, h2 = h // 2, h % 2
            ub = ubpool.tile([P, N], BF16)
            for half in range(2):
                ubps = pp_b.tile([P, 512], F32, tag="b", name=f"ubps{half}")
                nc.tensor.matmul(
                    ubps, ones_row, u_sb[h][:, half * 512 : (half + 1) * 512],
                    start=True, stop=True,
                )
                nc.scalar.activation(
                    ub[:, half * 512 : (half + 1) * 512], ubps, ACTF.Copy
                )
            pso = [
                pp_out.tile([FOUT + 1, 512], F32, tag="out", name=f"pso{half}")
                for half in range(2)
            ]
            for jc2 in range(NCH // 2):
                mx = mxpool.tile([P, 2, N], BF16)
                for k in range(2):
                    jc = 2 * jc2 + k
                    nc.vector.tensor_scalar(
                        mx[:, k, :], ub,
                        vq_col[:, p, jc, h2 : h2 + 1],
                        q_col[:, p, jc, h2 : h2 + 1],
                        ALU.mult, ALU.max,
                    )
                z = zpool.tile([P, 2, N], BF16)
                nc.vector.tensor_tensor(
                    z, mx, adjT[:, 2 * jc2 : 2 * jc2 + 2, :], ALU.mult
                )
                for k in range(2):
                    jc = 2 * jc2 + k
                    for half in range(2):
                        nc.tensor.matmul(
                            pso[half],
                            hp1[:, jc, h, :],
                            z[:, k, half * 512 : (half + 1) * 512],
                            start=(jc == 0),
                            stop=(jc == NCH - 1),
                        )
            ot = opool.tile([FOUT + 1, N], F32)
            nc.scalar.activation(ot[:, 0:512], pso[0], ACTF.Copy)
            nc.scalar.activation(ot[:, 512:N], pso[1], ACTF.Copy)
            nc.sync.dma_start(out_d[h], ot)


def build_program(num_devices=8, debug=False):
    nc = bacc.Bacc(
        "TRN2", target_bir_lowering=False, debug=debug, num_devices=num_devices
    )
    hT_d = nc.dram_tensor("hT", [KC, P, N], BF16, kind="ExternalInput").ap()
    w_d = nc.dram_tensor("w_all", [KC, P, NH * FOUT], BF16, kind="ExternalInput").ap()
    aPC_d = nc.dram_tensor("aPC", [NPAIR, P, 4], BF16, kind="ExternalInput").ap()
    adjT_d = nc.dram_tensor("adjT", [NCH, P, N], BF16, kind="ExternalInput").ap()
    out_d = nc.dram_tensor("outT", [NH, FOUT + 1, N], F32, kind="ExternalOutput").ap()
    with tile.TileContext(nc) as tc:
        emit(nc, tc, hT_d, w_d, aPC_d, adjT_d, out_d)
    nc.compile()
    return nc


def make_in_maps(h, adj, w, a_src, a_dst):
    """Host-side sharding/layout prep: core c gets batch c."""
    w_all = np.ascontiguousarray(
        w.astype(np.float32).transpose(1, 0, 2).reshape(KC, P, NH * FOUT)
    ).astype(BF16NP)
    # aPC[p]: [128, 4] = (src_A, src_B, dst_A, dst_B) columns for head pair
    # (2p, 2p+1); head A occupies partition rows 0:64, head B rows 64:128.
    aPC = np.zeros((NPAIR, P, 4), dtype=np.float32)
    for p in range(NPAIR):
        aPC[p, 0:FOUT, 0] = a_src[2 * p, :, 0]
        aPC[p, FOUT:P, 1] = a_src[2 * p + 1, :, 0]
        aPC[p, 0:FOUT, 2] = a_dst[2 * p, :, 0]
        aPC[p, FOUT:P, 3] = a_dst[2 * p + 1, :, 0]
    aPC = aPC.astype(BF16NP)
    in_maps = []
    for b in range(BS):
        hTb = np.ascontiguousarray(
            h[b].astype(np.float32).T.reshape(KC, P, N)
        ).astype(BF16NP)
        adjTb = np.ascontiguousarray(adj[b].T.reshape(NCH, P, N)).astype(BF16NP)
        in_maps.append({"hT": hTb, "w_all": w_all, "aPC": aPC, "adjT": adjTb})
    return in_maps


def postprocess(raw_outs):
    """raw_outs: list of [NH, FOUT+1, N] per core -> full [BS, NH, N, FOUT]."""
    outT = np.stack(raw_outs)  # [BS, NH, FOUT+1, N]
    num = outT[:, :, 0:FOUT, :]
    den = outT[:, :, FOUT : FOUT + 1, :]
    return np.ascontiguousarray((num / den).transpose(0, 1, 3, 2)).astype(np.float32)


_NC_CACHE = {}


def kernel(h, adj, w, a_src, a_dst):
    if "nc" not in _NC_CACHE:
        _NC_CACHE["nc"] = build_program(num_devices=BS)
    nc = _NC_CACHE["nc"]
    in_maps = make_in_maps(h, adj, w, a_src, a_dst)
    res = run_bass_kernel_spmd(nc, in_maps, core_ids=list(range(BS)))
    return postprocess([r["outT"] for r in res.results])
